# revision 11
# baseline (speedup 1.0000x reference)
"""DiscreteBipartiteFlow forward on 8 trn2 NeuronCores.

Math: inputs rows are exact one-hots (x0|x1). net = relu(x0@W1+b1)@W2+b2
only depends on i0=argmax(x0), so precompute (on device, per core) the
[V, 2V] table NET = relu(W1+b1)@W2+b2 and its per-row argmaxes
L[i]=argmax(NET[i,:V]), S[i]=argmax(NET[i,V:]). The straight-through
one_hot_argmax is numerically exactly-hard, one_hot_multiply of one-hots
is an index product, one_hot_add an index sum, so
z1 = one_hot((L[i0] + a1*S[i0]) mod V) (0 when S[i0]==0). Out = [x0|z1].

v2 layout (per core, 1024 rows, 8 rows per partition, DMA-lean):
 - 6 load DMAs (w1t+b1 fused / w2 / consts / b2 / input halves), all
   multi-KB descriptors, from the otherwise-idle SP sequencer.
 - table: relu+bias, NET accumulated on PE (bias row via ones-matmul),
   argmax via max/max_index, pack column pk128 = 128*(L + 128*S +
   16384*[S==0]) transposed to a row (PE transpose) and broadcast to all
   partitions with a ones-matmul.
 - data side: per row-slot the x1 half dotted with iota (mult + reduce)
   extracts a1 while the table runs; after the table, the x0 half dotted
   with the replicated pack row gives comb = 128*pack[i0] + a1 per row,
   already in [P, slot] layout -- no per-slot PE transposes or matmuls.
 - int32 unpack (power-of-2 masks/shifts), fold S==0 into an
   out-of-range compare index, one broadcast is_equal per chunk builds
   z1 IN PLACE over the x1 half, one fused store per chunk (x0
   passthrough rides along, 2KB descriptors).
Data-parallel over 8 cores; weights replicated.
"""

import numpy as np

V = 128
H = 512
N_CORES = 8
P = 128
NJ = 8               # row slots per partition
NCH = 4              # store chunks
CJ = NJ // NCH

# Feature toggles (HW-validated one at a time; see session notes)
USE_TTR = False        # tensor_tensor_reduce for the dots
USE_ACT = False        # relu + psum copies on the Activation engine
USE_M1MM = False       # M=1 matmul for pack-column -> row


def build_bass(rows: int):
    """Build the single-core Bass program for a [rows, 2V] batch shard."""
    import concourse.bacc as bacc
    import concourse.bass as bass
    import concourse.tile as tile
    from concourse import mybir

    f32 = mybir.dt.float32
    i32 = mybir.dt.int32
    u32 = mybir.dt.uint32
    A = mybir.AluOpType
    AF = mybir.ActivationFunctionType

    assert rows == P * NJ
    kh = H // P

    nc = bacc.Bacc(None)
    x = nc.declare_dram_parameter("x", [rows, 2 * V], f32, isOutput=False)
    # wf: per-partition fused [w1t (kh*V) | b1 (kh)]
    #   wf[p, k*V + i] = W1[i, k*P + p];  wf[p, kh*V + k] = b1[k*P + p]
    wf = nc.declare_dram_parameter("wf", [P, kh * V + kh], f32, isOutput=False)
    w2 = nc.declare_dram_parameter("w2", [H, 2 * V], f32, isOutput=False)
    b2 = nc.declare_dram_parameter("b2", [1, 2 * V], f32, isOutput=False)
    # host constants: slot0 = iota, slot1 = identity, slot2 = ones
    cst = nc.declare_dram_parameter("cst", [P, 3, V], f32, isOutput=False)
    out = nc.declare_dram_parameter("out", [rows, 2 * V], f32, isOutput=True)

    x_r = x.rearrange("(p j) n -> p j n", j=NJ)
    out_r = out.rearrange("(p j) n -> p j n", j=NJ)

    def bcast_mid(t_ap, reps):
        return bass.AP(
            tensor=t_ap.tensor, offset=t_ap.offset,
            ap=[t_ap.ap[0], [0, reps]] + list(t_ap.ap[1:]),
        )

    def bcast_last(t_ap, reps):
        return bass.AP(
            tensor=t_ap.tensor, offset=t_ap.offset,
            ap=list(t_ap.ap) + [[0, reps]],
        )

    with tile.TileContext(nc) as tc:
        with (
            tc.tile_pool(name="tab", bufs=1) as tab,
            tc.tile_pool(name="loop", bufs=1) as loop,
            tc.tile_pool(name="scr", bufs=2) as scr,
            tc.tile_pool(name="ps_net", bufs=1, space="PSUM") as ps_net,
            tc.tile_pool(name="ps_row", bufs=2, space="PSUM") as ps_row,
        ):
            # ---- t0: queue all loads on the SP sequencer ----
            wf_sb = tab.tile([P, kh * V + kh], f32)
            nc.sync.dma_start(out=wf_sb, in_=wf[:, :])
            w2_sb = tab.tile([P, kh, 2 * V], f32)
            nc.sync.dma_start(out=w2_sb, in_=w2.rearrange("(k p) n -> p k n", p=P))
            cst_sb = tab.tile([P, 3, V], f32)
            nc.sync.dma_start(out=cst_sb, in_=cst[:, :, :])
            b2_sb = tab.tile([1, 2 * V], f32)
            nc.sync.dma_start(out=b2_sb, in_=b2[:, :])
            xt = loop.tile([P, NJ, 2 * V], f32)
            nc.sync.dma_start(out=xt[:, 0 : NJ // 2, :], in_=x_r[:, 0 : NJ // 2, :])
            nc.sync.dma_start(out=xt[:, NJ // 2 :, :], in_=x_r[:, NJ // 2 :, :])

            w1t_sb = wf_sb[:, 0 : kh * V].rearrange("p (k i) -> p k i", k=kh)
            b1_sb = wf_sb[:, kh * V :]
            iota_f = cst_sb[:, 0, :]
            ident = cst_sb[:, 1, :]
            ones_pv = cst_sb[:, 2, :]

            if USE_ACT:
                act_warm = tab.tile([1, 1], f32)
                nc.scalar.activation(act_warm, ones_pv[0:1, 0:1], AF.Relu)

            # ---- a1 per slot from the x1 half (DVE, overlaps table) ----
            a1f = tab.tile([P, NJ], f32)
            for j in range(NJ):
                if USE_TTR:
                    pr = scr.tile([P, V], f32, tag="a1scr", bufs=2)
                    nc.vector.tensor_tensor_reduce(
                        out=pr, in0=xt[:, j, V :], in1=iota_f, scale=1.0,
                        scalar=0.0, op0=A.mult, op1=A.add,
                        accum_out=a1f[:, j : j + 1],
                    )
                else:
                    pr = scr.tile([P, V], f32, tag="a1scr", bufs=2)
                    nc.vector.tensor_mul(pr, xt[:, j, V :], iota_f)
                    nc.vector.reduce_sum(
                        a1f[:, j : j + 1], pr, axis=mybir.AxisListType.X
                    )

            # ---- table: NET = relu(W1+b1) @ W2 + b2 on [P=i0, 2V] ----
            hT = tab.tile([P, kh, V], f32)
            for k in range(kh):
                if USE_ACT:
                    nc.scalar.activation(
                        hT[:, k, :], w1t_sb[:, k, :], AF.Relu,
                        bias=b1_sb[:, k : k + 1], scale=1.0,
                    )
                else:
                    nc.vector.tensor_scalar(
                        out=hT[:, k, :], in0=w1t_sb[:, k, :],
                        scalar1=b1_sb[:, k : k + 1], scalar2=0.0,
                        op0=A.add, op1=A.max,
                    )
            net_ps = ps_net.tile([P, 2 * V], f32)
            for k in range(kh):
                nc.tensor.matmul(
                    net_ps, lhsT=hT[:, k, :], rhs=w2_sb[:, k, :],
                    start=(k == 0), stop=False,
                )
            nc.tensor.matmul(
                net_ps, lhsT=ones_pv[0:1, :], rhs=b2_sb, start=False, stop=True
            )
            net_sb = tab.tile([P, 2 * V], f32)
            if USE_ACT:
                nc.scalar.copy(net_sb, net_ps)
            else:
                nc.vector.tensor_copy(net_sb, net_ps)

            # argmax per head -> ix2[:, head, 0]
            ix2 = tab.tile([P, 2, 8], u32)
            for head in (0, 1):
                seg = net_sb[:, head * V : (head + 1) * V]
                m8 = tab.tile([P, 8], f32, tag=f"m8{head}")
                nc.vector.max(m8, seg)
                nc.vector.max_index(ix2[:, head, :], m8, seg)

            # pk128 = 128*(L + 128*S + 16384*[S==0]) as a [P, 1] column
            lfsf = tab.tile([P, 2], f32)
            nc.vector.tensor_copy(lfsf, ix2[:, :, 0])
            lf, sf = lfsf[:, 0:1], lfsf[:, 1:2]
            zf = tab.tile([P, 1], f32)
            nc.vector.tensor_scalar(out=zf, in0=sf, scalar1=0.5, scalar2=None, op0=A.is_le)
            pk = tab.tile([P, 1], f32)
            nc.vector.tensor_scalar(out=pk, in0=sf, scalar1=float(V), scalar2=lf, op0=A.mult, op1=A.add)
            pk2 = tab.tile([P, 1], f32)
            nc.vector.tensor_scalar(out=pk2, in0=zf, scalar1=float(V * V), scalar2=pk, op0=A.mult, op1=A.add)

            # pack column -> row -> replicated [P, V]
            if USE_M1MM:
                pk128 = tab.tile([P, 1], f32)
                nc.vector.tensor_scalar(out=pk128, in0=pk2, scalar1=float(V), scalar2=None, op0=A.mult)
                pkrow_ps = ps_row.tile([1, V], f32, tag="pkrow")
                nc.tensor.matmul(pkrow_ps, lhsT=pk128, rhs=ident, start=True, stop=True)
                pkrow_sb = tab.tile([1, V], f32)
                nc.vector.tensor_copy(pkrow_sb, pkrow_ps)
            else:
                # pad pack column into col 0 of a [P, V] tile, PE-transpose it
                pkpad = tab.tile([P, V], f32)
                nc.vector.tensor_scalar(out=pkpad[:, 0:1], in0=pk2, scalar1=float(V), scalar2=None, op0=A.mult)
                pkT_ps = ps_row.tile([P, V], f32, tag="pkT")
                nc.tensor.transpose(pkT_ps, pkpad, ident)
                pkrow_sb = tab.tile([1, V], f32)
                nc.vector.tensor_copy(pkrow_sb, pkT_ps[0:1, :])
            crep_ps = ps_row.tile([P, V], f32, tag="crep")
            nc.tensor.matmul(crep_ps, lhsT=ones_pv[0:1, :], rhs=pkrow_sb, start=True, stop=True)
            crep = tab.tile([P, V], f32)
            if USE_ACT:
                nc.scalar.copy(crep, crep_ps)
            else:
                nc.vector.tensor_copy(crep, crep_ps)

            # ---- comb = 128*pack[i0] + a1 per slot ----
            comb_f = tab.tile([P, NJ], f32)
            for j in range(NJ):
                if USE_TTR:
                    pr = scr.tile([P, V], f32, tag="lkscr", bufs=2)
                    nc.vector.tensor_tensor_reduce(
                        out=pr, in0=xt[:, j, 0:V], in1=crep, scale=1.0,
                        scalar=a1f[:, j : j + 1], op0=A.mult, op1=A.add,
                        accum_out=comb_f[:, j : j + 1],
                    )
                else:
                    pr = scr.tile([P, V], f32, tag="lkscr", bufs=2)
                    nc.vector.tensor_mul(pr, xt[:, j, 0:V], crep)
                    nc.vector.reduce_sum(
                        comb_f[:, j : j + 1], pr, axis=mybir.AxisListType.X
                    )
            if not USE_TTR:
                nc.vector.tensor_add(comb_f, comb_f, a1f)

            # ---- unpack: c = (S*a1 + L) & 127 | 256*[S==0] ----
            combi = tab.tile([P, NJ], i32)
            nc.vector.tensor_copy(combi, comb_f)
            a1i = tab.tile([P, NJ], i32)
            nc.vector.tensor_scalar(out=a1i, in0=combi, scalar1=V - 1, scalar2=None, op0=A.bitwise_and)
            wi = tab.tile([P, NJ], i32)
            nc.vector.tensor_scalar(out=wi, in0=combi, scalar1=7, scalar2=None, op0=A.arith_shift_right)
            li = tab.tile([P, NJ], i32)
            nc.vector.tensor_scalar(out=li, in0=wi, scalar1=V - 1, scalar2=None, op0=A.bitwise_and)
            s2 = tab.tile([P, NJ], i32)
            nc.vector.tensor_scalar(out=s2, in0=wi, scalar1=7, scalar2=None, op0=A.arith_shift_right)
            si = tab.tile([P, NJ], i32)
            nc.vector.tensor_scalar(out=si, in0=s2, scalar1=V - 1, scalar2=None, op0=A.bitwise_and)
            kill = tab.tile([P, NJ], i32)
            nc.vector.tensor_scalar(out=kill, in0=s2, scalar1=V, scalar2=1, op0=A.bitwise_and, op1=A.arith_shift_left)
            ti = tab.tile([P, NJ], i32)
            nc.vector.tensor_mul(ti, si, a1i)
            nc.vector.tensor_add(ti, ti, li)
            ci = tab.tile([P, NJ], i32)
            nc.vector.tensor_scalar(out=ci, in0=ti, scalar1=V - 1, scalar2=None, op0=A.bitwise_and)
            nc.vector.tensor_tensor(out=ci, in0=ci, in1=kill, op=A.bitwise_or)
            cf = tab.tile([P, NJ], f32)
            nc.vector.tensor_copy(cf, ci)

            # ---- z1 in place over the x1 half + fused store per chunk ----
            for ch in range(NCH):
                js = ch * CJ
                nc.vector.tensor_tensor(
                    out=xt[:, js : js + CJ, V :],
                    in0=bcast_mid(iota_f, CJ),
                    in1=bcast_last(cf[:, js : js + CJ], V),
                    op=A.is_equal,
                )
                nc.sync.dma_start(
                    out=out_r[:, js : js + CJ, :], in_=xt[:, js : js + CJ, :]
                )

    nc.finalize()
    return nc


# Test-harness hooks: extra kwargs for run_bass_kernel_spmd (e.g. trace=True)
# and the last BassKernelResults for profiling. Unused when graded.
RUN_KWARGS: dict = {}
LAST_RESULTS = None


def kernel(**inputs) -> np.ndarray:
    global LAST_RESULTS
    from concourse.bass_utils import run_bass_kernel_spmd

    x = np.ascontiguousarray(np.asarray(inputs["inputs"], dtype=np.float32))
    W1 = np.asarray(inputs["W1"], dtype=np.float32)
    kh = H // P
    # w1t[p, k, i] = W1[i, k*P + p] -- pure layout marshalling
    w1t = W1.T.reshape(kh, P, V).transpose(1, 0, 2).reshape(P, kh * V)
    b1c = np.asarray(inputs["b1"], dtype=np.float32).reshape(kh, P).T
    wf = np.ascontiguousarray(np.concatenate([w1t, b1c], axis=1))
    W2 = np.ascontiguousarray(np.asarray(inputs["W2"], dtype=np.float32))
    b2 = np.ascontiguousarray(np.asarray(inputs["b2"], dtype=np.float32).reshape(1, 2 * V))

    cstn = np.zeros((P, 3, V), np.float32)
    cstn[:, 0, :] = np.arange(V, dtype=np.float32)
    cstn[:, 1, :] = np.eye(V, dtype=np.float32)
    cstn[:, 2, :] = 1.0

    B = x.shape[0]
    rows = B // N_CORES
    nc = build_bass(rows)

    shards = np.split(x, N_CORES, axis=0)
    in_maps = [{"x": s, "wf": wf, "w2": W2, "b2": b2, "cst": cstn} for s in shards]
    res = run_bass_kernel_spmd(nc, in_maps, list(range(N_CORES)), **RUN_KWARGS)
    LAST_RESULTS = res
    return np.concatenate([r["out"] for r in res.results], axis=0)


# revision 14
# speedup vs baseline: 1.0739x; 1.0739x over previous
"""DiscreteBipartiteFlow forward on 8 trn2 NeuronCores.

Math: inputs rows are exact one-hots (x0|x1). net = relu(x0@W1+b1)@W2+b2
only depends on i0=argmax(x0), so precompute (on device, per core) the
[V, 2V] table NET = relu(W1+b1)@W2+b2 and its per-row argmaxes
L[i]=argmax(NET[i,:V]), S[i]=argmax(NET[i,V:]). The straight-through
one_hot_argmax is numerically exactly-hard, one_hot_multiply of one-hots
is an index product, one_hot_add an index sum, so
z1 = one_hot((L[i0] + a1*S[i0]) mod V) (0 when S[i0]==0). Out = [x0|z1].

v3 (per core, 1024 rows, 8 rows per partition). The real-HW profile is
descriptor-rate-bound on DMA (~90ns/descriptor/queue regardless of
size) and latency-bound on the dependent-op chain, so:
 - 3 load DMAs, 384 descriptors total: wfc fuses w1t|b1|iota|ident|
   ones|b2 into one [P, 1156] tensor (4.6KB/partition descriptors),
   w2 is host-permuted to [P, 4, 2V] (4KB descriptors), input loads as
   one [P, 8, 2V] DMA (8KB descriptors).
 - NET computed per head-half on PE so the loc-head argmax (max +
   max_index) overlaps the scale-head matmuls.
 - pack row trick: pk[i] = 256*L[i] + 32768*S[i] + 2^22*[S[i]==0] as a
   [P, 1] column, then crep = ones[P,P] @ (ident * pk) replicates it to
   every partition in ONE matmul (no transpose, no extra copies).
 - data side: one batched mult+reduce of the x1 half against iota
   (a1), one batched mult+reduce of the x0 half against crep;
   comb = crep[i0] + a1 per row, exact in fp32 (< 2^23).
 - int32 unpack (10 ops), z1 built in place over the x1 half via two
   broadcast is_equal chunks, two fused stores (4KB descriptors).
Data-parallel over 8 cores; weights replicated.
"""

import numpy as np

V = 128
H = 512
N_CORES = 8
P = 128
NJ = 8               # row slots per partition
NCH = 2              # store chunks
CJ = NJ // NCH

PSUM_MAX = True      # max/max_index read NET from PSUM directly

# wfc field offsets (f32 elements per partition)
OF_W1T = 0
OF_B1 = 512
OF_IOTA = 516
OF_IDENT = 644
OF_ONES = 772
OF_B2 = 900
WFC_W = 900 + 2 * V  # 1156


def build_bass(rows: int):
    """Build the single-core Bass program for a [rows, 2V] batch shard."""
    import concourse.bacc as bacc
    import concourse.bass as bass
    import concourse.tile as tile
    from concourse import mybir

    f32 = mybir.dt.float32
    i32 = mybir.dt.int32
    u32 = mybir.dt.uint32
    A = mybir.AluOpType
    AF = mybir.ActivationFunctionType

    assert rows == P * NJ
    kh = H // P

    nc = bacc.Bacc(None)
    x = nc.declare_dram_parameter("x", [rows, 2 * V], f32, isOutput=False)
    wfc = nc.declare_dram_parameter("wfc", [P, WFC_W], f32, isOutput=False)
    w2m = nc.declare_dram_parameter("w2m", [P, kh, 2 * V], f32, isOutput=False)
    out = nc.declare_dram_parameter("out", [rows, 2 * V], f32, isOutput=True)

    x_r = x.rearrange("(p j) n -> p j n", j=NJ)
    out_r = out.rearrange("(p j) n -> p j n", j=NJ)

    def bcast_mid(t_ap, reps):
        return bass.AP(
            tensor=t_ap.tensor, offset=t_ap.offset,
            ap=[t_ap.ap[0], [0, reps]] + list(t_ap.ap[1:]),
        )

    def bcast_last(t_ap, reps):
        return bass.AP(
            tensor=t_ap.tensor, offset=t_ap.offset,
            ap=list(t_ap.ap) + [[0, reps]],
        )

    with tile.TileContext(nc) as tc:
        with (
            tc.tile_pool(name="tab", bufs=1) as tab,
            tc.tile_pool(name="loop", bufs=1) as loop,
            tc.tile_pool(name="ps_net", bufs=2, space="PSUM") as ps_net,
            tc.tile_pool(name="ps_row", bufs=1, space="PSUM") as ps_row,
        ):
            # ---- t0: 3 load DMAs on the SP sequencer ----
            wfc_sb = tab.tile([P, WFC_W], f32)
            nc.sync.dma_start(out=wfc_sb, in_=wfc[:, :])
            w2_sb = tab.tile([P, kh, 2 * V], f32)
            nc.sync.dma_start(out=w2_sb, in_=w2m[:, :, :])
            xt = loop.tile([P, NJ, 2 * V], f32)
            nc.sync.dma_start(out=xt, in_=x_r[:, :, :])

            w1t_sb = wfc_sb[:, OF_W1T : OF_W1T + kh * V].rearrange(
                "p (k i) -> p k i", k=kh
            )
            b1_sb = wfc_sb[:, OF_B1 : OF_B1 + kh]
            iota_f = wfc_sb[:, OF_IOTA : OF_IOTA + V]
            ident = wfc_sb[:, OF_IDENT : OF_IDENT + V]
            ones_pv = wfc_sb[:, OF_ONES : OF_ONES + V]

            # ---- table: NET = relu(W1+b1) @ W2 + b2, per head-half ----
            hT = tab.tile([P, kh, V], f32)
            for k in range(kh):
                nc.scalar.activation(
                    hT[:, k, :], w1t_sb[:, k, :], AF.Relu,
                    bias=b1_sb[:, k : k + 1], scale=1.0,
                )
            halves = []
            for head in (0, 1):
                ps = ps_net.tile([P, V], f32, tag=f"net{head}")
                for k in range(kh):
                    nc.tensor.matmul(
                        ps, lhsT=hT[:, k, :],
                        rhs=w2_sb[:, k, head * V : (head + 1) * V],
                        start=(k == 0), stop=False,
                    )
                nc.tensor.matmul(
                    ps, lhsT=ones_pv[0:1, :],
                    rhs=wfc_sb[0:1, OF_B2 + head * V : OF_B2 + (head + 1) * V],
                    start=False, stop=True,
                )
                halves.append(ps)

            # argmax per head (overlaps the other half's matmuls)
            ix2 = tab.tile([P, 2, 8], u32)
            for head in (0, 1):
                if PSUM_MAX:
                    seg = halves[head]
                else:
                    seg = tab.tile([P, V], f32, tag=f"nsb{head}")
                    nc.scalar.copy(seg, halves[head])
                m8 = tab.tile([P, 8], f32, tag=f"m8{head}")
                nc.vector.max(m8, seg)
                nc.vector.max_index(ix2[:, head, :], m8, seg)

            # ---- a1 per row from the x1 half (batched mult + reduce) ----
            a1f = tab.tile([P, NJ], f32)
            a1s = loop.tile([P, NJ, V], f32, tag="a1scr")
            nc.vector.tensor_mul(a1s, xt[:, :, V :], bcast_mid(iota_f, NJ))
            nc.vector.reduce_sum(a1f, a1s, axis=mybir.AxisListType.X)

            # ---- pack column pk = 256L + 32768S + 2^22*[S==0] ----
            lfsf = tab.tile([P, 2], f32)
            nc.vector.tensor_copy(lfsf, ix2[:, :, 0])
            lf, sf = lfsf[:, 0:1], lfsf[:, 1:2]
            zf = tab.tile([P, 1], f32)
            nc.vector.tensor_scalar(out=zf, in0=sf, scalar1=0.5, scalar2=None, op0=A.is_le)
            q1 = tab.tile([P, 1], f32)
            nc.vector.tensor_scalar(out=q1, in0=sf, scalar1=float(V), scalar2=lf, op0=A.mult, op1=A.add)
            q256 = tab.tile([P, 1], f32)
            nc.vector.tensor_scalar(out=q256, in0=q1, scalar1=256.0, scalar2=None, op0=A.mult)
            pkfin = tab.tile([P, 1], f32)
            nc.vector.tensor_scalar(out=pkfin, in0=zf, scalar1=float(1 << 22), scalar2=q256, op0=A.mult, op1=A.add)

            # crep[p, i] = pk[i]: diag trick, one matmul
            diagp = tab.tile([P, V], f32)
            nc.vector.tensor_scalar(out=diagp, in0=ident, scalar1=pkfin, scalar2=None, op0=A.mult)
            crep_ps = ps_row.tile([P, V], f32)
            nc.tensor.matmul(crep_ps, lhsT=ones_pv, rhs=diagp, start=True, stop=True)
            crep = tab.tile([P, V], f32)
            nc.scalar.copy(crep, crep_ps)

            # ---- comb = crep[i0] + a1 per row (batched mult + reduce) ----
            comb_f = tab.tile([P, NJ], f32)
            lks = loop.tile([P, NJ, V], f32, tag="lkscr")
            nc.vector.tensor_mul(lks, xt[:, :, 0:V], bcast_mid(crep, NJ))
            nc.vector.reduce_sum(comb_f, lks, axis=mybir.AxisListType.X)
            nc.vector.tensor_add(comb_f, comb_f, a1f)

            # ---- unpack: c = (S*a1 + L) & 127 | 256*[S==0] ----
            combi = tab.tile([P, NJ], i32)
            nc.vector.tensor_copy(combi, comb_f)
            a1i = tab.tile([P, NJ], i32)
            nc.vector.tensor_scalar(out=a1i, in0=combi, scalar1=V - 1, scalar2=None, op0=A.bitwise_and)
            li = tab.tile([P, NJ], i32)
            nc.vector.tensor_scalar(out=li, in0=combi, scalar1=8, scalar2=V - 1, op0=A.arith_shift_right, op1=A.bitwise_and)
            si = tab.tile([P, NJ], i32)
            nc.vector.tensor_scalar(out=si, in0=combi, scalar1=15, scalar2=V - 1, op0=A.arith_shift_right, op1=A.bitwise_and)
            kill = tab.tile([P, NJ], i32)
            nc.vector.tensor_scalar(out=kill, in0=combi, scalar1=14, scalar2=2 * V, op0=A.arith_shift_right, op1=A.bitwise_and)
            ti = tab.tile([P, NJ], i32)
            nc.vector.tensor_mul(ti, si, a1i)
            nc.vector.tensor_add(ti, ti, li)
            ci = tab.tile([P, NJ], i32)
            nc.vector.tensor_scalar(out=ci, in0=ti, scalar1=V - 1, scalar2=None, op0=A.bitwise_and)
            nc.vector.tensor_tensor(out=ci, in0=ci, in1=kill, op=A.bitwise_or)
            cf = tab.tile([P, NJ], f32)
            nc.vector.tensor_copy(cf, ci)

            # ---- z1 in place over the x1 half + fused store per chunk ----
            for ch in range(NCH):
                js = ch * CJ
                nc.vector.tensor_tensor(
                    out=xt[:, js : js + CJ, V :],
                    in0=bcast_mid(iota_f, CJ),
                    in1=bcast_last(cf[:, js : js + CJ], V),
                    op=A.is_equal,
                )
                nc.sync.dma_start(
                    out=out_r[:, js : js + CJ, :], in_=xt[:, js : js + CJ, :]
                )

    nc.finalize()
    return nc


def _host_wfc(W1, b1, b2) -> np.ndarray:
    kh = H // P
    wfc = np.zeros((P, WFC_W), np.float32)
    # w1t[p, k*V + i] = W1[i, k*P + p] -- pure layout marshalling
    wfc[:, OF_W1T : OF_W1T + kh * V] = (
        W1.T.reshape(kh, P, V).transpose(1, 0, 2).reshape(P, kh * V)
    )
    wfc[:, OF_B1 : OF_B1 + kh] = b1.reshape(kh, P).T
    wfc[:, OF_IOTA : OF_IOTA + V] = np.arange(V, dtype=np.float32)
    wfc[:, OF_IDENT : OF_IDENT + V] = np.eye(V, dtype=np.float32)
    wfc[:, OF_ONES : OF_ONES + V] = 1.0
    wfc[:, OF_B2 : OF_B2 + 2 * V] = b2.reshape(1, 2 * V)
    return np.ascontiguousarray(wfc)


# Test-harness hooks: extra kwargs for run_bass_kernel_spmd (e.g. trace=True)
# and the last BassKernelResults for profiling. Unused when graded.
RUN_KWARGS: dict = {}
LAST_RESULTS = None


def kernel(**inputs) -> np.ndarray:
    global LAST_RESULTS
    from concourse.bass_utils import run_bass_kernel_spmd

    x = np.ascontiguousarray(np.asarray(inputs["inputs"], dtype=np.float32))
    W1 = np.asarray(inputs["W1"], dtype=np.float32)
    b1 = np.asarray(inputs["b1"], dtype=np.float32)
    W2 = np.asarray(inputs["W2"], dtype=np.float32)
    b2 = np.asarray(inputs["b2"], dtype=np.float32)
    kh = H // P
    wfc = _host_wfc(W1, b1, b2)
    # w2m[p, k, :] = W2[k*P + p, :] -- pure layout marshalling
    w2m = np.ascontiguousarray(W2.reshape(kh, P, 2 * V).transpose(1, 0, 2))

    B = x.shape[0]
    rows = B // N_CORES
    nc = build_bass(rows)

    shards = np.split(x, N_CORES, axis=0)
    in_maps = [{"x": s, "wfc": wfc, "w2m": w2m} for s in shards]
    res = run_bass_kernel_spmd(nc, in_maps, list(range(N_CORES)), **RUN_KWARGS)
    LAST_RESULTS = res
    return np.concatenate([r["out"] for r in res.results], axis=0)


# revision 15
# speedup vs baseline: 1.0914x; 1.0163x over previous
"""DiscreteBipartiteFlow forward on 8 trn2 NeuronCores.

Math: inputs rows are exact one-hots (x0|x1). net = relu(x0@W1+b1)@W2+b2
only depends on i0=argmax(x0), so precompute (on device, per core) the
[V, 2V] table NET = relu(W1+b1)@W2+b2 and its per-row argmaxes
L[i]=argmax(NET[i,:V]), S[i]=argmax(NET[i,V:]). The straight-through
one_hot_argmax is numerically exactly-hard, one_hot_multiply of one-hots
is an index product, one_hot_add an index sum, so
z1 = one_hot((L[i0] + a1*S[i0]) mod V) (0 when S[i0]==0). Out = [x0|z1].

v4 (per core, 1024 rows, 8 rows per partition). Real-HW profile is
bytes-bound on DMA loads and latency/DVE-bound on the dependent chain:
 - 5 load DMAs ordered by need: w1t+b1 / w2-scale-half / w2-loc-half
   (host-permuted so each half is contiguous per partition) / consts
   (iota|ident|ones|b2) / input (one DMA, 8KB descriptors).
 - scale head computed FIRST on PE so its argmax + pack chain overlaps
   the loc-head matmuls; b2 matmuls are skipped when b2 == 0 (checked
   on host; spec fill is zeros) else added via ones-matmul.
 - pack row pk[i] = 256*L[i] + 32768*S[i] + 2^22*[S[i]==0] replicated
   to all partitions via TWO PSUM-accumulated diag matmuls
   (crep = ones @ (ident*packS) + ones @ (ident*packL)) -- the S part
   runs before the loc argmax is even known.
 - data side: one batched mult+reduce of the x1 half vs iota (a1), one
   of the x0 half vs crep; comb = pk[i0] + a1, exact in fp32 (< 2^23).
 - int32 unpack (10 ops), z1 built in place over the x1 half via two
   broadcast is_equal chunks, two fused stores (4KB descriptors).
Data-parallel over 8 cores; weights replicated.
"""

import numpy as np

V = 128
H = 512
N_CORES = 8
P = 128
NJ = 8               # row slots per partition
NCH = 2              # store chunks
CJ = NJ // NCH

# cst field offsets (f32 elements per partition)
OF_IOTA = 0
OF_IDENT = V
OF_ONES = 2 * V
OF_B2 = 3 * V
CST_W = 5 * V


def build_bass(rows: int, use_b2: bool):
    """Build the single-core Bass program for a [rows, 2V] batch shard."""
    import concourse.bacc as bacc
    import concourse.bass as bass
    import concourse.tile as tile
    from concourse import mybir

    f32 = mybir.dt.float32
    i32 = mybir.dt.int32
    u32 = mybir.dt.uint32
    A = mybir.AluOpType
    AF = mybir.ActivationFunctionType

    assert rows == P * NJ
    kh = H // P

    nc = bacc.Bacc(None)
    x = nc.declare_dram_parameter("x", [rows, 2 * V], f32, isOutput=False)
    # wa: per-partition fused [w1t (kh*V) | b1 (kh)]
    wa = nc.declare_dram_parameter("wa", [P, kh * V + kh], f32, isOutput=False)
    # w2h[p, h, k*V + c] = W2[k*P + p, h*V + c]; h=0 scale head, h=1 loc head
    w2h = nc.declare_dram_parameter("w2h", [P, 2, kh * V], f32, isOutput=False)
    cst = nc.declare_dram_parameter("cst", [P, CST_W], f32, isOutput=False)
    out = nc.declare_dram_parameter("out", [rows, 2 * V], f32, isOutput=True)

    x_r = x.rearrange("(p j) n -> p j n", j=NJ)
    out_r = out.rearrange("(p j) n -> p j n", j=NJ)

    def bcast_mid(t_ap, reps):
        return bass.AP(
            tensor=t_ap.tensor, offset=t_ap.offset,
            ap=[t_ap.ap[0], [0, reps]] + list(t_ap.ap[1:]),
        )

    def bcast_last(t_ap, reps):
        return bass.AP(
            tensor=t_ap.tensor, offset=t_ap.offset,
            ap=list(t_ap.ap) + [[0, reps]],
        )

    with tile.TileContext(nc) as tc:
        with (
            tc.tile_pool(name="tab", bufs=1) as tab,
            tc.tile_pool(name="loop", bufs=1) as loop,
            tc.tile_pool(name="ps_net", bufs=2, space="PSUM") as ps_net,
            tc.tile_pool(name="ps_row", bufs=1, space="PSUM") as ps_row,
        ):
            # ---- t0: load DMAs on the SP sequencer, in order of need ----
            wa_sb = tab.tile([P, kh * V + kh], f32)
            nc.sync.dma_start(out=wa_sb, in_=wa[:, :])
            w2_sb = tab.tile([P, 2, kh, V], f32)
            w2h_r = w2h.rearrange("p h (k c) -> p h k c", k=kh)
            nc.sync.dma_start(out=w2_sb[:, 0], in_=w2h_r[:, 0])
            nc.sync.dma_start(out=w2_sb[:, 1], in_=w2h_r[:, 1])
            cst_sb = tab.tile([P, CST_W], f32)
            nc.sync.dma_start(out=cst_sb, in_=cst[:, :])
            xt = loop.tile([P, NJ, 2 * V], f32)
            nc.sync.dma_start(out=xt, in_=x_r[:, :, :])

            w1t_sb = wa_sb[:, 0 : kh * V].rearrange("p (k i) -> p k i", k=kh)
            b1_sb = wa_sb[:, kh * V :]
            iota_f = cst_sb[:, OF_IOTA : OF_IOTA + V]
            ident = cst_sb[:, OF_IDENT : OF_IDENT + V]
            ones_pv = cst_sb[:, OF_ONES : OF_ONES + V]

            # ---- table: NET = relu(W1+b1) @ W2 (+ b2), scale head first ----
            hT = tab.tile([P, kh, V], f32)
            for k in range(kh):
                nc.scalar.activation(
                    hT[:, k, :], w1t_sb[:, k, :], AF.Relu,
                    bias=b1_sb[:, k : k + 1], scale=1.0,
                )
            halves = {}
            for h, head in ((0, 1), (1, 0)):  # h: w2h slot; head: 0=loc, 1=scale
                ps = ps_net.tile([P, V], f32, tag=f"net{head}")
                for k in range(kh):
                    last = k == kh - 1 and not use_b2
                    nc.tensor.matmul(
                        ps, lhsT=hT[:, k, :], rhs=w2_sb[:, h, k],
                        start=(k == 0), stop=last,
                    )
                if use_b2:
                    nc.tensor.matmul(
                        ps, lhsT=ones_pv[0:1, :],
                        rhs=cst_sb[0:1, OF_B2 + head * V : OF_B2 + (head + 1) * V],
                        start=False, stop=True,
                    )
                halves[head] = ps

            crep_ps = ps_row.tile([P, V], f32)

            # scale-head argmax + its half of the pack row
            ixS = tab.tile([P, 8], u32)
            m8S = tab.tile([P, 8], f32)
            nc.vector.max(m8S, halves[1])
            nc.vector.max_index(ixS, m8S, halves[1])
            sfT = tab.tile([P, 1], f32)
            nc.vector.tensor_copy(sfT, ixS[:, 0:1])
            zf = tab.tile([P, 1], f32)
            nc.vector.tensor_scalar(out=zf, in0=sfT, scalar1=0.5, scalar2=None, op0=A.is_le)
            qS = tab.tile([P, 1], f32)
            nc.vector.tensor_scalar(out=qS, in0=sfT, scalar1=32768.0, scalar2=None, op0=A.mult)
            qS2 = tab.tile([P, 1], f32)
            nc.vector.tensor_scalar(out=qS2, in0=zf, scalar1=float(1 << 22), scalar2=qS, op0=A.mult, op1=A.add)
            diagS = tab.tile([P, V], f32)
            nc.vector.tensor_scalar(out=diagS, in0=ident, scalar1=qS2, scalar2=None, op0=A.mult)
            nc.tensor.matmul(crep_ps, lhsT=ones_pv, rhs=diagS, start=True, stop=False)

            # loc-head argmax + its half of the pack row
            ixL = tab.tile([P, 8], u32)
            m8L = tab.tile([P, 8], f32)
            nc.vector.max(m8L, halves[0])
            nc.vector.max_index(ixL, m8L, halves[0])
            qL = tab.tile([P, 1], f32)
            nc.vector.tensor_copy(qL, ixL[:, 0:1])
            diagL = tab.tile([P, V], f32)
            nc.vector.tensor_scalar(out=diagL, in0=ident, scalar1=qL, scalar2=None, op0=A.mult)
            dL256 = tab.tile([P, V], f32)
            nc.vector.tensor_scalar(out=dL256, in0=diagL, scalar1=256.0, scalar2=None, op0=A.mult)
            nc.tensor.matmul(crep_ps, lhsT=ones_pv, rhs=dL256, start=False, stop=True)
            crep = tab.tile([P, V], f32)
            nc.scalar.copy(crep, crep_ps)

            # ---- a1 per row from the x1 half (batched mult + reduce) ----
            a1f = tab.tile([P, NJ], f32)
            a1s = loop.tile([P, NJ, V], f32, tag="a1scr")
            nc.vector.tensor_mul(a1s, xt[:, :, V :], bcast_mid(iota_f, NJ))
            nc.vector.reduce_sum(a1f, a1s, axis=mybir.AxisListType.X)

            # ---- comb = pk[i0] + a1 per row (batched mult + reduce) ----
            comb_f = tab.tile([P, NJ], f32)
            lks = loop.tile([P, NJ, V], f32, tag="lkscr")
            nc.vector.tensor_mul(lks, xt[:, :, 0:V], bcast_mid(crep, NJ))
            nc.vector.reduce_sum(comb_f, lks, axis=mybir.AxisListType.X)
            nc.vector.tensor_add(comb_f, comb_f, a1f)

            # ---- unpack: c = (S*a1 + L) & 127 | 256*[S==0] ----
            combi = tab.tile([P, NJ], i32)
            nc.vector.tensor_copy(combi, comb_f)
            a1i = tab.tile([P, NJ], i32)
            nc.vector.tensor_scalar(out=a1i, in0=combi, scalar1=V - 1, scalar2=None, op0=A.bitwise_and)
            li = tab.tile([P, NJ], i32)
            nc.vector.tensor_scalar(out=li, in0=combi, scalar1=8, scalar2=V - 1, op0=A.arith_shift_right, op1=A.bitwise_and)
            si = tab.tile([P, NJ], i32)
            nc.vector.tensor_scalar(out=si, in0=combi, scalar1=15, scalar2=V - 1, op0=A.arith_shift_right, op1=A.bitwise_and)
            kill = tab.tile([P, NJ], i32)
            nc.vector.tensor_scalar(out=kill, in0=combi, scalar1=14, scalar2=2 * V, op0=A.arith_shift_right, op1=A.bitwise_and)
            ti = tab.tile([P, NJ], i32)
            nc.vector.tensor_mul(ti, si, a1i)
            nc.vector.tensor_add(ti, ti, li)
            ci = tab.tile([P, NJ], i32)
            nc.vector.tensor_scalar(out=ci, in0=ti, scalar1=V - 1, scalar2=None, op0=A.bitwise_and)
            nc.vector.tensor_tensor(out=ci, in0=ci, in1=kill, op=A.bitwise_or)
            cf = tab.tile([P, NJ], f32)
            nc.vector.tensor_copy(cf, ci)

            # ---- z1 in place over the x1 half + fused store per chunk ----
            for ch in range(NCH):
                js = ch * CJ
                nc.vector.tensor_tensor(
                    out=xt[:, js : js + CJ, V :],
                    in0=bcast_mid(iota_f, CJ),
                    in1=bcast_last(cf[:, js : js + CJ], V),
                    op=A.is_equal,
                )
                nc.sync.dma_start(
                    out=out_r[:, js : js + CJ, :], in_=xt[:, js : js + CJ, :]
                )

    nc.finalize()
    return nc


def _host_wa(W1, b1) -> np.ndarray:
    kh = H // P
    wa = np.zeros((P, kh * V + kh), np.float32)
    # w1t[p, k*V + i] = W1[i, k*P + p] -- pure layout marshalling
    wa[:, 0 : kh * V] = W1.T.reshape(kh, P, V).transpose(1, 0, 2).reshape(P, kh * V)
    wa[:, kh * V :] = b1.reshape(kh, P).T
    return np.ascontiguousarray(wa)


def _host_cst(b2) -> np.ndarray:
    cst = np.zeros((P, CST_W), np.float32)
    cst[:, OF_IOTA : OF_IOTA + V] = np.arange(V, dtype=np.float32)
    cst[:, OF_IDENT : OF_IDENT + V] = np.eye(V, dtype=np.float32)
    cst[:, OF_ONES : OF_ONES + V] = 1.0
    cst[:, OF_B2 : OF_B2 + 2 * V] = b2.reshape(1, 2 * V)
    return np.ascontiguousarray(cst)


def _host_w2h(W2) -> np.ndarray:
    kh = H // P
    # w2h[p, h, k*V + c] = W2[k*P + p, h*V + c]; h=0 scale, h=1 loc
    w4 = W2.reshape(kh, P, 2, V)             # [k, p, h(loc=0/scale=1), c]
    w2h = w4.transpose(1, 2, 0, 3)[:, ::-1]  # [p, h(scale,loc), k, c]
    return np.ascontiguousarray(w2h.reshape(P, 2, kh * V))


# Test-harness hooks: extra kwargs for run_bass_kernel_spmd (e.g. trace=True)
# and the last BassKernelResults for profiling. Unused when graded.
RUN_KWARGS: dict = {}
LAST_RESULTS = None


def kernel(**inputs) -> np.ndarray:
    global LAST_RESULTS
    from concourse.bass_utils import run_bass_kernel_spmd

    x = np.ascontiguousarray(np.asarray(inputs["inputs"], dtype=np.float32))
    W1 = np.asarray(inputs["W1"], dtype=np.float32)
    b1 = np.asarray(inputs["b1"], dtype=np.float32)
    W2 = np.asarray(inputs["W2"], dtype=np.float32)
    b2 = np.asarray(inputs["b2"], dtype=np.float32)
    use_b2 = bool(np.any(b2 != 0.0))

    wa = _host_wa(W1, b1)
    w2hn = _host_w2h(W2)
    cstn = _host_cst(b2)

    B = x.shape[0]
    rows = B // N_CORES
    nc = build_bass(rows, use_b2)

    shards = np.split(x, N_CORES, axis=0)
    in_maps = [{"x": s, "wa": wa, "w2h": w2hn, "cst": cstn} for s in shards]
    res = run_bass_kernel_spmd(nc, in_maps, list(range(N_CORES)), **RUN_KWARGS)
    LAST_RESULTS = res
    return np.concatenate([r["out"] for r in res.results], axis=0)


# revision 16
# speedup vs baseline: 1.1373x; 1.0421x over previous
"""DiscreteBipartiteFlow forward on 8 trn2 NeuronCores.

Math: inputs rows are exact one-hots (x0|x1). net = relu(x0@W1+b1)@W2+b2
only depends on i0=argmax(x0), so precompute (on device, per core) the
[V, 2V] table NET = relu(W1+b1)@W2+b2 and its per-row argmaxes
L[i]=argmax(NET[i,:V]), S[i]=argmax(NET[i,V:]). The straight-through
one_hot_argmax is numerically exactly-hard, one_hot_multiply of one-hots
is an index product, one_hot_add an index sum, so
z1 = one_hot((L[i0] + a1*S[i0]) mod V) (0 when S[i0]==0). Out = [x0|z1].

v5 (per core, 1024 rows, 8 rows per partition). Real-HW profile is
descriptor-count-bound on DMA (~200ns/descriptor/queue for 2-8KB) and
DVE/latency-bound on the dependent chain:
 - 3 load DMAs / 384 descriptors, fused by when they're needed:
   A = w1t|b1|w2-scale-half (4.1KB/partition), B = iota|ident|ones|b2|
   w2-loc-half (4.6KB), C = input (8KB). w2 halves host-permuted to be
   contiguous per partition.
 - scale head computed FIRST on PE so its argmax+pack chain overlaps
   the loc-head matmuls; b2 matmuls skipped when b2 == 0 (host-checked;
   spec fill is zeros).
 - pack row pk[i] = 256*L[i] + 32768*S[i] + 2^22*[S[i]==0] replicated
   to all partitions via TWO PSUM-accumulated diag matmuls
   (crep = ones @ (ident*packS) + ones @ (ident*packL)); the lookup
   mult reads crep straight from PSUM (no copy). Table-side DVE ops run
   under high_priority so the scheduler doesn't queue the (long) a1
   reduction ahead of them.
 - data side: one batched mult+reduce of the x1 half vs iota (a1), one
   of the x0 half vs crep; comb = pk[i0] + a1, exact in fp32 (< 2^23).
 - int32 unpack (10 ops), z1 built in place over the x1 half via two
   broadcast is_equal chunks, two fused stores (4KB descriptors).
Data-parallel over 8 cores; weights replicated.
"""

import numpy as np

V = 128
H = 512
N_CORES = 8
P = 128
NJ = 8               # row slots per partition
NCH = 2              # store chunks
CJ = NJ // NCH

KH = H // P
# tensor A field offsets (f32 elements per partition): w1t | b1 | w2scale
A_W1T = 0
A_B1 = KH * V
A_W2S = KH * V + KH
A_W = KH * V + KH + KH * V
# tensor B field offsets: iota | ident | ones | b2 | w2loc
B_IOTA = 0
B_IDENT = V
B_ONES = 2 * V
B_B2 = 3 * V
B_W2L = 5 * V
B_W = 5 * V + KH * V


def build_bass(rows: int, use_b2: bool):
    """Build the single-core Bass program for a [rows, 2V] batch shard."""
    import concourse.bacc as bacc
    import concourse.bass as bass
    import concourse.tile as tile
    from concourse import mybir

    f32 = mybir.dt.float32
    i32 = mybir.dt.int32
    u32 = mybir.dt.uint32
    A = mybir.AluOpType
    AF = mybir.ActivationFunctionType

    assert rows == P * NJ

    nc = bacc.Bacc(None)
    x = nc.declare_dram_parameter("x", [rows, 2 * V], f32, isOutput=False)
    ta = nc.declare_dram_parameter("ta", [P, A_W], f32, isOutput=False)
    tb = nc.declare_dram_parameter("tb", [P, B_W], f32, isOutput=False)
    out = nc.declare_dram_parameter("out", [rows, 2 * V], f32, isOutput=True)

    x_r = x.rearrange("(p j) n -> p j n", j=NJ)
    out_r = out.rearrange("(p j) n -> p j n", j=NJ)

    def bcast_mid(t_ap, reps):
        return bass.AP(
            tensor=t_ap.tensor, offset=t_ap.offset,
            ap=[t_ap.ap[0], [0, reps]] + list(t_ap.ap[1:]),
        )

    def bcast_last(t_ap, reps):
        return bass.AP(
            tensor=t_ap.tensor, offset=t_ap.offset,
            ap=list(t_ap.ap) + [[0, reps]],
        )

    with tile.TileContext(nc) as tc:
        with (
            tc.tile_pool(name="tab", bufs=1) as tab,
            tc.tile_pool(name="loop", bufs=1) as loop,
            tc.tile_pool(name="ps_net", bufs=2, space="PSUM") as ps_net,
            tc.tile_pool(name="ps_row", bufs=1, space="PSUM") as ps_row,
        ):
            # ---- t0: 3 load DMAs on the SP sequencer, in order of need ----
            ta_sb = tab.tile([P, A_W], f32)
            nc.sync.dma_start(out=ta_sb, in_=ta[:, :])
            tb_sb = tab.tile([P, B_W], f32)
            nc.sync.dma_start(out=tb_sb, in_=tb[:, :])
            xt = loop.tile([P, NJ, 2 * V], f32)
            nc.sync.dma_start(out=xt, in_=x_r[:, :, :])

            w1t_sb = ta_sb[:, A_W1T : A_W1T + KH * V].rearrange("p (k i) -> p k i", k=KH)
            b1_sb = ta_sb[:, A_B1 : A_B1 + KH]
            w2s = ta_sb[:, A_W2S : A_W2S + KH * V].rearrange("p (k c) -> p k c", k=KH)
            iota_f = tb_sb[:, B_IOTA : B_IOTA + V]
            ident = tb_sb[:, B_IDENT : B_IDENT + V]
            ones_pv = tb_sb[:, B_ONES : B_ONES + V]
            w2l = tb_sb[:, B_W2L : B_W2L + KH * V].rearrange("p (k c) -> p k c", k=KH)

            # ---- table: NET = relu(W1+b1) @ W2 (+ b2), scale head first ----
            hT = tab.tile([P, KH, V], f32)
            for k in range(KH):
                nc.scalar.activation(
                    hT[:, k, :], w1t_sb[:, k, :], AF.Relu,
                    bias=b1_sb[:, k : k + 1], scale=1.0,
                )
            halves = {}
            for head, w2half in ((1, w2s), (0, w2l)):  # 1=scale first, 0=loc
                ps = ps_net.tile([P, V], f32, tag=f"net{head}")
                for k in range(KH):
                    last = k == KH - 1 and not use_b2
                    nc.tensor.matmul(
                        ps, lhsT=hT[:, k, :], rhs=w2half[:, k],
                        start=(k == 0), stop=last,
                    )
                if use_b2:
                    nc.tensor.matmul(
                        ps, lhsT=ones_pv[0:1, :],
                        rhs=tb_sb[0:1, B_B2 + head * V : B_B2 + (head + 1) * V],
                        start=False, stop=True,
                    )
                halves[head] = ps

            crep_ps = ps_row.tile([P, V], f32)

            with tc.high_priority():
                # scale-head argmax + its half of the pack row
                ixS = tab.tile([P, 8], u32)
                m8S = tab.tile([P, 8], f32)
                nc.vector.max(m8S, halves[1])
                nc.vector.max_index(ixS, m8S, halves[1])
                sfT = tab.tile([P, 1], f32)
                nc.vector.tensor_copy(sfT, ixS[:, 0:1])
                zf = tab.tile([P, 1], f32)
                nc.vector.tensor_scalar(out=zf, in0=sfT, scalar1=0.5, scalar2=None, op0=A.is_le)
                qS = tab.tile([P, 1], f32)
                nc.vector.tensor_scalar(out=qS, in0=sfT, scalar1=32768.0, scalar2=None, op0=A.mult)
                qS2 = tab.tile([P, 1], f32)
                nc.vector.tensor_scalar(out=qS2, in0=zf, scalar1=float(1 << 22), scalar2=qS, op0=A.mult, op1=A.add)
                diagS = tab.tile([P, V], f32)
                nc.vector.tensor_scalar(out=diagS, in0=ident, scalar1=qS2, scalar2=None, op0=A.mult)
                nc.tensor.matmul(crep_ps, lhsT=ones_pv, rhs=diagS, start=True, stop=False)

                # loc-head argmax + its half of the pack row
                ixL = tab.tile([P, 8], u32)
                m8L = tab.tile([P, 8], f32)
                nc.vector.max(m8L, halves[0])
                nc.vector.max_index(ixL, m8L, halves[0])
                qL = tab.tile([P, 1], f32)
                nc.vector.tensor_copy(qL, ixL[:, 0:1])
                diagL = tab.tile([P, V], f32)
                nc.vector.tensor_scalar(out=diagL, in0=ident, scalar1=qL, scalar2=256.0, op0=A.mult, op1=A.mult)
                nc.tensor.matmul(crep_ps, lhsT=ones_pv, rhs=diagL, start=False, stop=True)

            # ---- a1 per row from the x1 half (batched mult + reduce) ----
            a1f = tab.tile([P, NJ], f32)
            a1s = loop.tile([P, NJ, V], f32, tag="a1scr")
            nc.vector.tensor_mul(a1s, xt[:, :, V :], bcast_mid(iota_f, NJ))
            nc.vector.reduce_sum(a1f, a1s, axis=mybir.AxisListType.X)

            # ---- comb = pk[i0] + a1 per row (batched mult + reduce) ----
            comb_f = tab.tile([P, NJ], f32)
            lks = loop.tile([P, NJ, V], f32, tag="lkscr")
            nc.vector.tensor_mul(lks, xt[:, :, 0:V], bcast_mid(crep_ps, NJ))
            nc.vector.reduce_sum(comb_f, lks, axis=mybir.AxisListType.X)
            nc.vector.tensor_add(comb_f, comb_f, a1f)

            # ---- unpack: c = (S*a1 + L) & 127 | 256*[S==0] ----
            combi = tab.tile([P, NJ], i32)
            nc.vector.tensor_copy(combi, comb_f)
            a1i = tab.tile([P, NJ], i32)
            nc.vector.tensor_scalar(out=a1i, in0=combi, scalar1=V - 1, scalar2=None, op0=A.bitwise_and)
            li = tab.tile([P, NJ], i32)
            nc.vector.tensor_scalar(out=li, in0=combi, scalar1=8, scalar2=V - 1, op0=A.arith_shift_right, op1=A.bitwise_and)
            si = tab.tile([P, NJ], i32)
            nc.vector.tensor_scalar(out=si, in0=combi, scalar1=15, scalar2=V - 1, op0=A.arith_shift_right, op1=A.bitwise_and)
            kill = tab.tile([P, NJ], i32)
            nc.vector.tensor_scalar(out=kill, in0=combi, scalar1=14, scalar2=2 * V, op0=A.arith_shift_right, op1=A.bitwise_and)
            ti = tab.tile([P, NJ], i32)
            nc.vector.tensor_mul(ti, si, a1i)
            nc.vector.tensor_add(ti, ti, li)
            ci = tab.tile([P, NJ], i32)
            nc.vector.tensor_scalar(out=ci, in0=ti, scalar1=V - 1, scalar2=None, op0=A.bitwise_and)
            nc.vector.tensor_tensor(out=ci, in0=ci, in1=kill, op=A.bitwise_or)
            cf = tab.tile([P, NJ], f32)
            nc.vector.tensor_copy(cf, ci)

            # ---- z1 in place over the x1 half + fused store per chunk ----
            for ch in range(NCH):
                js = ch * CJ
                nc.vector.tensor_tensor(
                    out=xt[:, js : js + CJ, V :],
                    in0=bcast_mid(iota_f, CJ),
                    in1=bcast_last(cf[:, js : js + CJ], V),
                    op=A.is_equal,
                )
                nc.sync.dma_start(
                    out=out_r[:, js : js + CJ, :], in_=xt[:, js : js + CJ, :]
                )

    nc.finalize()
    return nc


def _host_w2_halves(W2):
    # w2half[p, k*V + c] = W2[k*P + p, half*V + c]; returns (scale, loc)
    w4 = W2.reshape(KH, P, 2, V).transpose(1, 2, 0, 3)  # [p, half, k, c]
    loc = w4[:, 0].reshape(P, KH * V)
    scale = w4[:, 1].reshape(P, KH * V)
    return scale, loc


def _host_ta(W1, b1, W2) -> np.ndarray:
    ta = np.zeros((P, A_W), np.float32)
    # w1t[p, k*V + i] = W1[i, k*P + p] -- pure layout marshalling
    ta[:, A_W1T : A_W1T + KH * V] = (
        W1.T.reshape(KH, P, V).transpose(1, 0, 2).reshape(P, KH * V)
    )
    ta[:, A_B1 : A_B1 + KH] = b1.reshape(KH, P).T
    ta[:, A_W2S : A_W2S + KH * V] = _host_w2_halves(W2)[0]
    return np.ascontiguousarray(ta)


def _host_tb(W2, b2) -> np.ndarray:
    tb = np.zeros((P, B_W), np.float32)
    tb[:, B_IOTA : B_IOTA + V] = np.arange(V, dtype=np.float32)
    tb[:, B_IDENT : B_IDENT + V] = np.eye(V, dtype=np.float32)
    tb[:, B_ONES : B_ONES + V] = 1.0
    tb[:, B_B2 : B_B2 + 2 * V] = b2.reshape(1, 2 * V)
    tb[:, B_W2L : B_W2L + KH * V] = _host_w2_halves(W2)[1]
    return np.ascontiguousarray(tb)


# Test-harness hooks: extra kwargs for run_bass_kernel_spmd (e.g. trace=True)
# and the last BassKernelResults for profiling. Unused when graded.
RUN_KWARGS: dict = {}
LAST_RESULTS = None


def kernel(**inputs) -> np.ndarray:
    global LAST_RESULTS
    from concourse.bass_utils import run_bass_kernel_spmd

    x = np.ascontiguousarray(np.asarray(inputs["inputs"], dtype=np.float32))
    W1 = np.asarray(inputs["W1"], dtype=np.float32)
    b1 = np.asarray(inputs["b1"], dtype=np.float32)
    W2 = np.asarray(inputs["W2"], dtype=np.float32)
    b2 = np.asarray(inputs["b2"], dtype=np.float32)
    use_b2 = bool(np.any(b2 != 0.0))

    tan = _host_ta(W1, b1, W2)
    tbn = _host_tb(W2, b2)

    B = x.shape[0]
    rows = B // N_CORES
    nc = build_bass(rows, use_b2)

    shards = np.split(x, N_CORES, axis=0)
    in_maps = [{"x": s, "ta": tan, "tb": tbn} for s in shards]
    res = run_bass_kernel_spmd(nc, in_maps, list(range(N_CORES)), **RUN_KWARGS)
    LAST_RESULTS = res
    return np.concatenate([r["out"] for r in res.results], axis=0)


# revision 17
# speedup vs baseline: 1.2264x; 1.0784x over previous
"""DiscreteBipartiteFlow forward on 8 trn2 NeuronCores.

Math: inputs rows are exact one-hots (x0|x1). net = relu(x0@W1+b1)@W2+b2
only depends on i0=argmax(x0), so precompute (on device, per core) the
[V, 2V] table NET = relu(W1+b1)@W2+b2 and its per-row argmaxes
L[i]=argmax(NET[i,:V]), S[i]=argmax(NET[i,V:]). The straight-through
one_hot_argmax is numerically exactly-hard, one_hot_multiply of one-hots
is an index product, one_hot_add an index sum, so
z1 = one_hot((L[i0] + a1*S[i0]) mod V) (0 when S[i0]==0). Out = [x0|z1].

v5 (per core, 1024 rows, 8 rows per partition). Real-HW profile is
descriptor-count-bound on DMA (~200ns/descriptor/queue for 2-8KB) and
DVE/latency-bound on the dependent chain:
 - 3 load DMAs / 384 descriptors, fused by when they're needed:
   A = w1t|b1|w2-scale-half (4.1KB/partition), B = iota|ident|ones|b2|
   w2-loc-half (4.6KB), C = input (8KB). w2 halves host-permuted to be
   contiguous per partition.
 - scale head computed FIRST on PE so its argmax+pack chain overlaps
   the loc-head matmuls; b2 matmuls skipped when b2 == 0 (host-checked;
   spec fill is zeros).
 - pack row pk[i] = 256*L[i] + 32768*S[i] + 2^22*[S[i]==0] replicated
   to all partitions via TWO PSUM-accumulated diag matmuls
   (crep = ones @ (ident*packS) + ones @ (ident*packL)); the lookup
   mult reads crep straight from PSUM (no copy). Table-side DVE ops run
   under high_priority so the scheduler doesn't queue the (long) a1
   reduction ahead of them.
 - data side: one batched mult+reduce of the x1 half vs iota (a1), one
   of the x0 half vs crep; comb = pk[i0] + a1, exact in fp32 (< 2^23).
 - int32 unpack (10 ops), z1 built in place over the x1 half via two
   broadcast is_equal chunks, two fused stores (4KB descriptors).
Data-parallel over 8 cores; weights replicated.
"""

import numpy as np

V = 128
H = 512
N_CORES = 8
P = 128
NJ = 8               # row slots per partition
NCH = 2              # store chunks
CJ = NJ // NCH

KH = H // P
# tensor A field offsets (f32 elements per partition): w1t | b1 | w2scale
A_W1T = 0
A_B1 = KH * V
A_W2S = KH * V + KH
A_W = KH * V + KH + KH * V
# tensor B field offsets: iota | ident | ones | b2 | w2loc
B_IOTA = 0
B_IDENT = V
B_ONES = 2 * V
B_B2 = 3 * V
B_W2L = 5 * V
B_W = 5 * V + KH * V


def build_bass(rows: int, use_b2: bool):
    """Build the single-core Bass program for a [rows, 2V] batch shard."""
    import concourse.bacc as bacc
    import concourse.bass as bass
    import concourse.tile as tile
    from concourse import mybir

    f32 = mybir.dt.float32
    i32 = mybir.dt.int32
    u32 = mybir.dt.uint32
    A = mybir.AluOpType
    AF = mybir.ActivationFunctionType

    assert rows == P * NJ

    nc = bacc.Bacc(None)
    x = nc.declare_dram_parameter("x", [rows, 2 * V], f32, isOutput=False)
    ta = nc.declare_dram_parameter("ta", [P, A_W], f32, isOutput=False)
    tb = nc.declare_dram_parameter("tb", [P, B_W], f32, isOutput=False)
    out = nc.declare_dram_parameter("out", [rows, 2 * V], f32, isOutput=True)

    x_r = x.rearrange("(p j) n -> p j n", j=NJ)
    out_r = out.rearrange("(p j) n -> p j n", j=NJ)

    def bcast_mid(t_ap, reps):
        return bass.AP(
            tensor=t_ap.tensor, offset=t_ap.offset,
            ap=[t_ap.ap[0], [0, reps]] + list(t_ap.ap[1:]),
        )

    def bcast_last(t_ap, reps):
        return bass.AP(
            tensor=t_ap.tensor, offset=t_ap.offset,
            ap=list(t_ap.ap) + [[0, reps]],
        )

    with tile.TileContext(nc) as tc:
        with (
            tc.tile_pool(name="tab", bufs=1) as tab,
            tc.tile_pool(name="loop", bufs=1) as loop,
            tc.tile_pool(name="ps_net", bufs=2, space="PSUM") as ps_net,
            tc.tile_pool(name="ps_row", bufs=1, space="PSUM") as ps_row,
        ):
            # ---- t0: 3 load DMAs on the SP sequencer, in order of need ----
            ta_sb = tab.tile([P, A_W], f32)
            nc.sync.dma_start(out=ta_sb, in_=ta[:, :])
            tb_sb = tab.tile([P, B_W], f32)
            nc.sync.dma_start(out=tb_sb, in_=tb[:, :])
            xt = loop.tile([P, NJ, 2 * V], f32)
            nc.sync.dma_start(out=xt, in_=x_r[:, :, :])

            w1t_sb = ta_sb[:, A_W1T : A_W1T + KH * V].rearrange("p (k i) -> p k i", k=KH)
            b1_sb = ta_sb[:, A_B1 : A_B1 + KH]
            w2s = ta_sb[:, A_W2S : A_W2S + KH * V].rearrange("p (k c) -> p k c", k=KH)
            iota_f = tb_sb[:, B_IOTA : B_IOTA + V]
            ident = tb_sb[:, B_IDENT : B_IDENT + V]
            ones_pv = tb_sb[:, B_ONES : B_ONES + V]
            w2l = tb_sb[:, B_W2L : B_W2L + KH * V].rearrange("p (k c) -> p k c", k=KH)

            # ---- table: NET = relu(W1+b1) @ W2 (+ b2), scale head first ----
            hT = tab.tile([P, KH, V], f32)
            for k in range(KH):
                nc.scalar.activation(
                    hT[:, k, :], w1t_sb[:, k, :], AF.Relu,
                    bias=b1_sb[:, k : k + 1], scale=1.0,
                )
            halves = {}
            for head, w2half in ((1, w2s), (0, w2l)):  # 1=scale first, 0=loc
                ps = ps_net.tile([P, V], f32, tag=f"net{head}")
                for k in range(KH):
                    last = k == KH - 1 and not use_b2
                    nc.tensor.matmul(
                        ps, lhsT=hT[:, k, :], rhs=w2half[:, k],
                        start=(k == 0), stop=last,
                    )
                if use_b2:
                    nc.tensor.matmul(
                        ps, lhsT=ones_pv[0:1, :],
                        rhs=tb_sb[0:1, B_B2 + head * V : B_B2 + (head + 1) * V],
                        start=False, stop=True,
                    )
                halves[head] = ps

            crep_ps = ps_row.tile([P, V], f32)

            with tc.high_priority():
                # scale-head argmax + its half of the pack row
                ixS = tab.tile([P, 8], u32)
                m8S = tab.tile([P, 8], f32)
                nc.vector.max(m8S, halves[1])
                nc.vector.max_index(ixS, m8S, halves[1])
                sfT = tab.tile([P, 1], f32)
                nc.vector.tensor_copy(sfT, ixS[:, 0:1])
                zf = tab.tile([P, 1], f32)
                nc.vector.tensor_scalar(out=zf, in0=sfT, scalar1=0.5, scalar2=None, op0=A.is_le)
                qS = tab.tile([P, 1], f32)
                nc.vector.tensor_scalar(out=qS, in0=sfT, scalar1=32768.0, scalar2=None, op0=A.mult)
                qS2 = tab.tile([P, 1], f32)
                nc.vector.tensor_scalar(out=qS2, in0=zf, scalar1=float(1 << 22), scalar2=qS, op0=A.mult, op1=A.add)
                diagS = tab.tile([P, V], f32)
                nc.vector.tensor_scalar(out=diagS, in0=ident, scalar1=qS2, scalar2=None, op0=A.mult)
                nc.tensor.matmul(crep_ps, lhsT=ones_pv, rhs=diagS, start=True, stop=False)

                # loc-head argmax + its half of the pack row
                ixL = tab.tile([P, 8], u32)
                m8L = tab.tile([P, 8], f32)
                nc.vector.max(m8L, halves[0])
                nc.vector.max_index(ixL, m8L, halves[0])
                qL = tab.tile([P, 1], f32)
                nc.vector.tensor_copy(qL, ixL[:, 0:1])
                diagL = tab.tile([P, V], f32)
                nc.vector.tensor_scalar(out=diagL, in0=ident, scalar1=qL, scalar2=256.0, op0=A.mult, op1=A.mult)
                nc.tensor.matmul(crep_ps, lhsT=ones_pv, rhs=diagL, start=False, stop=True)

            # ---- a1 per row from the x1 half (batched mult + reduce) ----
            a1f = tab.tile([P, NJ], f32)
            a1s = loop.tile([P, NJ, V], f32, tag="a1scr")
            # WAW gate: keep the (long) a1 mult from being scheduled onto DVE
            # ahead of the table chain -- it would stall DVE waiting for x.
            nc.vector.tensor_copy(a1s[0:1, 0, 0:1], diagL[0:1, 0:1])
            nc.vector.tensor_mul(a1s, xt[:, :, V :], bcast_mid(iota_f, NJ))
            nc.vector.reduce_sum(a1f, a1s, axis=mybir.AxisListType.X)

            # ---- comb = pk[i0] + a1 per row (batched mult + reduce) ----
            comb_f = tab.tile([P, NJ], f32)
            lks = loop.tile([P, NJ, V], f32, tag="lkscr")
            nc.vector.tensor_mul(lks, xt[:, :, 0:V], bcast_mid(crep_ps, NJ))
            nc.vector.reduce_sum(comb_f, lks, axis=mybir.AxisListType.X)
            nc.vector.tensor_add(comb_f, comb_f, a1f)

            # ---- unpack: c = (S*a1 + L) & 127 | 256*[S==0] ----
            combi = tab.tile([P, NJ], i32)
            nc.vector.tensor_copy(combi, comb_f)
            a1i = tab.tile([P, NJ], i32)
            nc.vector.tensor_scalar(out=a1i, in0=combi, scalar1=V - 1, scalar2=None, op0=A.bitwise_and)
            li = tab.tile([P, NJ], i32)
            nc.vector.tensor_scalar(out=li, in0=combi, scalar1=8, scalar2=V - 1, op0=A.arith_shift_right, op1=A.bitwise_and)
            si = tab.tile([P, NJ], i32)
            nc.vector.tensor_scalar(out=si, in0=combi, scalar1=15, scalar2=V - 1, op0=A.arith_shift_right, op1=A.bitwise_and)
            kill = tab.tile([P, NJ], i32)
            nc.vector.tensor_scalar(out=kill, in0=combi, scalar1=14, scalar2=2 * V, op0=A.arith_shift_right, op1=A.bitwise_and)
            ti = tab.tile([P, NJ], i32)
            nc.vector.tensor_mul(ti, si, a1i)
            nc.vector.tensor_add(ti, ti, li)
            ci = tab.tile([P, NJ], i32)
            nc.vector.tensor_scalar(out=ci, in0=ti, scalar1=V - 1, scalar2=None, op0=A.bitwise_and)
            nc.vector.tensor_tensor(out=ci, in0=ci, in1=kill, op=A.bitwise_or)
            cf = tab.tile([P, NJ], f32)
            nc.vector.tensor_copy(cf, ci)

            # ---- z1 in place over the x1 half + fused store per chunk ----
            for ch in range(NCH):
                js = ch * CJ
                nc.vector.tensor_tensor(
                    out=xt[:, js : js + CJ, V :],
                    in0=bcast_mid(iota_f, CJ),
                    in1=bcast_last(cf[:, js : js + CJ], V),
                    op=A.is_equal,
                )
                nc.sync.dma_start(
                    out=out_r[:, js : js + CJ, :], in_=xt[:, js : js + CJ, :]
                )

    nc.finalize()
    return nc


def _host_w2_halves(W2):
    # w2half[p, k*V + c] = W2[k*P + p, half*V + c]; returns (scale, loc)
    w4 = W2.reshape(KH, P, 2, V).transpose(1, 2, 0, 3)  # [p, half, k, c]
    loc = w4[:, 0].reshape(P, KH * V)
    scale = w4[:, 1].reshape(P, KH * V)
    return scale, loc


def _host_ta(W1, b1, W2) -> np.ndarray:
    ta = np.zeros((P, A_W), np.float32)
    # w1t[p, k*V + i] = W1[i, k*P + p] -- pure layout marshalling
    ta[:, A_W1T : A_W1T + KH * V] = (
        W1.T.reshape(KH, P, V).transpose(1, 0, 2).reshape(P, KH * V)
    )
    ta[:, A_B1 : A_B1 + KH] = b1.reshape(KH, P).T
    ta[:, A_W2S : A_W2S + KH * V] = _host_w2_halves(W2)[0]
    return np.ascontiguousarray(ta)


def _host_tb(W2, b2) -> np.ndarray:
    tb = np.zeros((P, B_W), np.float32)
    tb[:, B_IOTA : B_IOTA + V] = np.arange(V, dtype=np.float32)
    tb[:, B_IDENT : B_IDENT + V] = np.eye(V, dtype=np.float32)
    tb[:, B_ONES : B_ONES + V] = 1.0
    tb[:, B_B2 : B_B2 + 2 * V] = b2.reshape(1, 2 * V)
    tb[:, B_W2L : B_W2L + KH * V] = _host_w2_halves(W2)[1]
    return np.ascontiguousarray(tb)


# Test-harness hooks: extra kwargs for run_bass_kernel_spmd (e.g. trace=True)
# and the last BassKernelResults for profiling. Unused when graded.
RUN_KWARGS: dict = {}
LAST_RESULTS = None


def kernel(**inputs) -> np.ndarray:
    global LAST_RESULTS
    from concourse.bass_utils import run_bass_kernel_spmd

    x = np.ascontiguousarray(np.asarray(inputs["inputs"], dtype=np.float32))
    W1 = np.asarray(inputs["W1"], dtype=np.float32)
    b1 = np.asarray(inputs["b1"], dtype=np.float32)
    W2 = np.asarray(inputs["W2"], dtype=np.float32)
    b2 = np.asarray(inputs["b2"], dtype=np.float32)
    use_b2 = bool(np.any(b2 != 0.0))

    tan = _host_ta(W1, b1, W2)
    tbn = _host_tb(W2, b2)

    B = x.shape[0]
    rows = B // N_CORES
    nc = build_bass(rows, use_b2)

    shards = np.split(x, N_CORES, axis=0)
    in_maps = [{"x": s, "ta": tan, "tb": tbn} for s in shards]
    res = run_bass_kernel_spmd(nc, in_maps, list(range(N_CORES)), **RUN_KWARGS)
    LAST_RESULTS = res
    return np.concatenate([r["out"] for r in res.results], axis=0)
